# revision 20
# baseline (speedup 1.0000x reference)
"""Trainium2 Bass kernel for nn_Embedding2Score (session-graph attention +
vocab-scored readout).

Sharding (8 NeuronCores):
  - phase 1 (attention + segment pooling): data-parallel over sessions —
    core k owns graphs [k*128, (k+1)*128) == nodes [k*6400, (k+1)*6400).
  - peer gather of the per-shard pooled vectors s_g^T (32KB/core): each
    core sends its s_g directly into the SBUF of every peer via
    remote_dma_broadcast (SWDGE prep + trigger, XOR-relative routing so
    the SPMD program needs no runtime addresses), receivers wait on a
    remote semaphore. This bypasses the collectives firmware whose
    fixed entry barrier (~66us) + op latency would gate the z phase.
  - phase 2 (z = s_h @ item_emb.T): tensor-parallel over the vocab V —
    core k owns item columns [k*12500, (k+1)*12500) and emits z[:, shard]
    (optionally plus E extra gather-free own-row tiles, see E).

Block mapping is XOR-relative: core k's local block j = global graph
block (k XOR j). The peer send with Delta=j lands in slot j on core
k XOR j, so slot j on core k holds s_g of core (k XOR j) — matching the
host-side vnf rotation and z un-rotation.

Layout facts:
  - All matmuls keep features on partitions so the natural [in,out]
    weight storage is lhsT with zero on-device transposes.
  - The v_n -> nodes broadcast is a DVE tensor_copy with a stride-0
    (broadcast) inner AP dim (each graph column repeated L=50 times).
  - alpha is computed as a ROW ([1, N]) via q^T @ S matmuls, broadcast
    to all partitions with gpsimd.partition_broadcast, and the ragged
    segment-sum becomes a single DVE windowed reduce (axis X over
    [H, Bs, L]) — no selector matrices and no second copy of X on HBM.
  - z is written bf16 (host upcasts); halves the dominant HBM write.
"""

from contextlib import ExitStack

import numpy as np

H = 128
B = 1024
L = 50
N = B * L
V = 100000
M = 8            # cores
Bs = B // M      # 128 graphs / core
Ns = N // M      # 6400 nodes / core
Vs = V // M      # 12500 vocab cols / core
CH = 512         # phase-1a chunk width (nodes)
ZG = 6250        # z output group width (vocab cols); 2 groups per tile
E = 3            # extra own-row vocab tiles per core (gather-free fill)
PEER_GATHER = False  # remote_dma peer gather crashes this runtime (NRT 101)


def _sigmoid(x):
    out = np.empty_like(x)
    np.negative(x, out=out)
    np.exp(out, out=out)
    out += 1.0
    np.reciprocal(out, out=out)
    return out


def _kernel_numpy(session, item, batch, W1, b1, W2, b2, q, bq, W3, b3):
    """General-batch fallback (host only). Handles any sorted batch."""
    nb = int(batch.max()) + 1
    last_idx = np.searchsorted(batch, np.arange(nb), side="right") - 1
    v_n = session[last_idx]
    pre = _sigmoid(v_n[batch] @ W1 + b1 + session @ W2 + b2)
    alpha = pre @ q + bq
    w = alpha * session
    s_g = np.zeros((nb, session.shape[1]), np.float32)
    np.add.at(s_g, batch, w)
    s_h = np.concatenate([v_n, s_g], axis=1) @ W3 + b3
    return (s_h @ item.T).astype(np.float32)


def _build_program(bq_val):
    import concourse.bass as bass
    import concourse.bacc as bacc
    import concourse.tile as tile
    from concourse import mybir

    F32 = mybir.dt.float32
    BF16 = mybir.dt.bfloat16
    SIG = mybir.ActivationFunctionType.Sigmoid
    IDN = mybir.ActivationFunctionType.Identity

    nc = bacc.Bacc("TRN2", target_bir_lowering=False, debug=False,
                   num_devices=M)

    # ---- DRAM I/O (per-core data; identical program on all cores) ----
    d_xT = nc.dram_tensor("xT", [H, Ns], BF16, kind="ExternalInput").ap()
    # v_n^T with columns XOR-rotated: local block j = global block (rank^j)
    d_vnf = nc.dram_tensor("vnf", [H, B], BF16, kind="ExternalInput").ap()
    # bf16 weights packed: [W1 | W2 | W3a | W3b | q] along the free dim
    d_wp = nc.dram_tensor("wp", [H, 4 * H + 1], BF16,
                          kind="ExternalInput").ap()
    # f32 biases packed: [b1+b2 | b3]
    d_bp = nc.dram_tensor("bp", [H, 2], F32, kind="ExternalInput").ap()
    # item columns: own shard + E following shards (rank^i for i<=E)
    HX = ZG if E == 3 else 0   # extra half-tile columns (E=3.5 scheme)
    d_item = nc.dram_tensor("itemT", [H, (1 + E) * Vs + HX], BF16,
                            kind="ExternalInput").ap()
    # z rows in LOCAL (XOR) block order; blocks j in 1..E are never
    # written (their row-owner computes them as extras; see d_ze).
    d_z = nc.dram_tensor("z", [B, Vs], BF16, kind="ExternalOutput").ap()
    if E > 0:
        d_ze = nc.dram_tensor("ze", [Bs, E * Vs + HX], BF16,
                              kind="ExternalOutput").ap()

    if not PEER_GATHER:
        cc_in = nc.dram_tensor("cc_in", [H, Bs], BF16).ap()
        cc_out = nc.dram_tensor("cc_out", [M * H, Bs], BF16,
                                addr_space="Shared").ap()

    rsem = nc.alloc_semaphore("peer_rsem")
    lsem = nc.alloc_semaphore("peer_lsem")

    with tile.TileContext(nc) as tc, ExitStack() as ctx:
        nc_ = tc.nc

        consts = ctx.enter_context(tc.tile_pool(name="consts", bufs=1))
        small = ctx.enter_context(tc.tile_pool(name="small", bufs=1))
        item_pool = ctx.enter_context(tc.tile_pool(name="itemp", bufs=1))
        work = ctx.enter_context(tc.tile_pool(name="work", bufs=3))
        big1 = ctx.enter_context(tc.tile_pool(name="big1", bufs=1))
        zout = ctx.enter_context(tc.tile_pool(name="zout", bufs=4))
        psum_a = ctx.enter_context(
            tc.tile_pool(name="psum_a", bufs=2, space="PSUM"))
        psum_q = ctx.enter_context(
            tc.tile_pool(name="psum_q", bufs=2, space="PSUM"))
        psum_z = ctx.enter_context(
            tc.tile_pool(name="psum_z", bufs=4, space="PSUM"))

        # ---- input loads; first item chunk rides the idle sync ring ----
        wp_sb = consts.tile([H, 4 * H + 1], BF16)
        bp_sb = consts.tile([H, 2], F32)
        vnf_sb = consts.tile([H, B], BF16)
        xT_sb = big1.tile([H, Ns], BF16)
        itemT_sb = item_pool.tile([H, (1 + E) * Vs + HX], BF16)

        half = Ns // 2
        # Urgent phase-1 inputs lead the scalar ring; ONLY those five go
        # on scalar — descriptor-issue instructions occupy the issuing
        # engine's queue, and once the ~9-semaphore pool wraps they WAIT
        # inline for an old descriptor to drain, which would block the
        # scalar activations behind them for ~25us. All item loads issue
        # from the otherwise-idle sync ring, in 1.6MB slices so the
        # per-descriptor engine round-robin can't starve the weights.
        # The weight loads head the SAME queue as the item slices so the
        # per-descriptor engine round-robin cannot starve them; xT rides
        # the scalar queue alone and only competes with the tiny weights.
        nc_.sync.dma_start(out=wp_sb[:], in_=d_wp[:])
        nc_.sync.dma_start(out=bp_sb[:], in_=d_bp[:])
        nc_.sync.dma_start(out=vnf_sb[:], in_=d_vnf[:])
        nc_.scalar.dma_start(out=xT_sb[:, :half], in_=d_xT[:, :half])
        nc_.scalar.dma_start(out=xT_sb[:, half:], in_=d_xT[:, half:])
        ISL = 6250          # item descriptor slice (1.6MB)
        ncols = (1 + E) * Vs + HX
        for c0 in range(0, ncols, ISL):
            c1 = min(c0 + ISL, ncols)
            nc_.sync.dma_start(out=itemT_sb[:, c0:c1], in_=d_item[:, c0:c1])

        w1s = wp_sb[:, 0 * H:1 * H]
        w2s = wp_sb[:, 1 * H:2 * H]
        w3as = wp_sb[:, 2 * H:3 * H]
        w3bs = wp_sb[:, 3 * H:4 * H]
        qs = wp_sb[:, 4 * H:4 * H + 1]
        bcs = bp_sb[:, 0:1]
        b3s = bp_sb[:, 1:2]

        # ---- phase 1 prologue: Av^T + bc, broadcast to nodes ----
        av_sb = small.tile([H, Bs], BF16)       # (v_n W1 + bc)^T, own graphs
        avrep = big1.tile([H, Ns], BF16)        # column g repeated L times
        p_av = psum_a.tile([H, CH], F32, tag="pp", name="p_av")
        nc_.tensor.matmul(p_av[:, :Bs], lhsT=w1s, rhs=vnf_sb[:, :Bs],
                          start=True, stop=True)
        nc_.scalar.activation(av_sb[:], p_av[:, :Bs], IDN, bias=bcs)
        nc_.vector.tensor_copy(
            out=avrep[:].rearrange("h (g l) -> h g l", g=Bs, l=L),
            in_=av_sb[:].unsqueeze(2).broadcast_to([H, Bs, L]))

        # ---- phase 1a: S = sigmoid(W2^T X^T + avrep); alpha row = q^T S
        # Chunks are emitted in PAIRS so the tensor queue runs
        # MM1,MM1,MMq,MMq — a lone MMq between MM1s would serialize the
        # whole add->sigmoid->q chain into the matmul cadence.
        alpharow = small.tile([1, Ns], BF16)
        n_chunks = (Ns + CH - 1) // CH
        c = 0
        while c < n_chunks:
            pair = []
            for cc2 in (c, c + 1):
                if cc2 >= n_chunks:
                    continue
                c0 = cc2 * CH
                cw = min(CH, Ns - c0)
                pp = psum_a.tile([H, CH], F32, tag="pp")
                nc_.tensor.matmul(pp[:, :cw], lhsT=w2s,
                                  rhs=xT_sb[:, c0:c0 + cw],
                                  start=True, stop=True)
                s_sb = work.tile([H, CH], BF16, tag="schunk")
                nc_.vector.tensor_add(s_sb[:, :cw], pp[:, :cw],
                                      avrep[:, c0:c0 + cw])
                nc_.scalar.activation(s_sb[:, :cw], s_sb[:, :cw], SIG)
                pair.append((c0, cw, s_sb))
            for c0, cw, s_sb in pair:
                pq = psum_q.tile([1, CH], F32, tag="pq")
                nc_.tensor.matmul(pq[:1, :cw], lhsT=qs, rhs=s_sb[:, :cw],
                                  start=True, stop=True)
                nc_.vector.tensor_scalar_add(alpharow[:, c0:c0 + cw],
                                             pq[:1, :cw], float(bq_val))
            c += 2

        # ---- phase 1c: Xw = X^T * alpha; s_g^T = windowed sum over L ----
        alpharep = avrep        # broadcast in place; avrep region q is
        xw_sb = xT_sb           # dead once quarter q's adds ran, and xT
                                # is dead after the in-place multiply
        sgf = small.tile([H, Bs], F32)
        sg_sb = small.tile([H, Bs], BF16)
        qq = Ns // 8
        for qi in range(8):
            s0, s1 = qi * qq, (qi + 1) * qq
            nc_.gpsimd.partition_broadcast(alpharep[:, s0:s1],
                                           alpharow[:, s0:s1])
            nc_.vector.tensor_mul(xw_sb[:, s0:s1], xT_sb[:, s0:s1],
                                  alpharep[:, s0:s1])
            nc_.vector.tensor_reduce(
                out=sgf[:, s0 // L:s1 // L],
                in_=xw_sb[:, s0:s1].rearrange("h (g l) -> h g l", l=L),
                axis=mybir.AxisListType.X, op=mybir.AluOpType.add)
        nc_.vector.tensor_copy(out=sg_sb[:], in_=sgf[:])

        # ---- gather every shard's s_g^T ----
        shT_sb = small.tile([H, B], BF16)

        def sh_block(j, rhs2):
            p_sh = psum_a.tile([H, CH], F32, tag="pp", name=f"p_sh{j}")
            nc_.tensor.matmul(p_sh[:, :Bs], lhsT=w3as,
                              rhs=vnf_sb[:, j * Bs:(j + 1) * Bs],
                              start=True, stop=False)
            nc_.tensor.matmul(p_sh[:, :Bs], lhsT=w3bs, rhs=rhs2,
                              start=False, stop=True)
            nc_.scalar.activation(shT_sb[:, j * Bs:(j + 1) * Bs],
                                  p_sh[:, :Bs], IDN, bias=b3s)

        if PEER_GATHER:
            # slot j-1 of gath <- s_g of core (rank ^ j), delivered by that
            # core's Delta=j send. Preps only write descriptors; the trigger
            # (which carries the RAW dep on sg_sb) fires them all.
            gath = small.tile([H, (M - 1) * Bs], BF16)
            gath_safe = small.tile([H, (M - 1) * Bs], BF16)
            for dlt in range(1, M):
                rd = [None] * M
                rd[dlt] = (0, dlt)
                nc_.gpsimd.remote_dma_broadcast(
                    gath[:, (dlt - 1) * Bs:dlt * Bs], sg_sb[:],
                    remote_sem=rsem, local_sem=lsem, rdests=rd)
            nc_.gpsimd.trigger_dma(count=None)
        else:
            nc_.sync.dma_start(out=cc_in[:], in_=sg_sb[:])
            nc_.gpsimd.collective_compute(
                "AllGather", mybir.AluOpType.bypass,
                replica_groups=[list(range(M))],
                ins=[cc_in.opt()], outs=[cc_out.opt()])

        eng_i = 0

        def z_tile(lhs, isrc, dst, ring_sel, groups=(0, 1), fine=False):
            """[128 rows] x groups of ZG cols from item cols isrc -> dst.
            fine=True splits each group's write in two so the kernel's
            final DMA isn't a fully-exposed 1.6MB drain."""
            nonlocal eng_i
            FS = 3072  # chunk-aligned split point within a group
            for g in groups:
                g0 = g * ZG
                zt = zout.tile([H, ZG], BF16, tag="zt")
                for u in range(0, ZG, CH):
                    uw = min(CH, ZG - u)
                    zp = psum_z.tile([H, CH], F32, tag="zp")
                    nc_.tensor.matmul(
                        zp[:, :uw], lhsT=lhs,
                        rhs=itemT_sb[:, isrc + g0 + u:isrc + g0 + u + uw],
                        start=True, stop=True)
                    if eng_i % 2 == 0:
                        nc_.vector.tensor_copy(out=zt[:, u:u + uw],
                                               in_=zp[:, :uw])
                    else:
                        nc_.scalar.copy(out=zt[:, u:u + uw], in_=zp[:, :uw])
                    eng_i += 1
                    if fine and u + uw == FS:
                        ring = nc_.gpsimd if ring_sel % 2 == 0 else nc_.sync
                        ring.dma_start(out=dst[:, g0:g0 + FS],
                                       in_=zt[:, :FS])
                        ring_sel += 1
                ring = nc_.gpsimd if ring_sel % 2 == 0 else nc_.sync
                lo = FS if fine else 0
                ring.dma_start(out=dst[:, g0 + lo:g0 + ZG], in_=zt[:, lo:])
                ring_sel += 1

        # own block + extra own-row tiles first: fully local, overlaps
        # the gather latency
        sh_block(0, sg_sb[:])
        z_tile(shT_sb[:, 0:H], 0, d_z[0:H, :], 0)
        for i in range(1, E + 1):
            z_tile(shT_sb[:, 0:H], i * Vs,
                   d_ze[:, (i - 1) * Vs:i * Vs], i)
        if HX:
            # half tile: own rows x first half of vocab shard (rank-4)
            z_tile(shT_sb[:, 0:H], (1 + E) * Vs,
                   d_ze[:, E * Vs:E * Vs + ZG], E + 1, groups=(0,))

        # remaining blocks read the gathered s_g; blocks j in 1..E are
        # owned (as own-row tiles) by their row cores.
        if PEER_GATHER:
            # The rsem wait is attached POST-scheduling (the Tile sim is
            # single-core and would deadlock on a remotely-incremented
            # sem). The memset->copy WAW edge on gath_safe orders the
            # copy after the waiting memset.
            rwait_inst = nc_.gpsimd.memset(gath_safe[0:1, 0:1], 0.0)
            nc_.gpsimd.tensor_copy(out=gath_safe[:], in_=gath[:])
            for j in range(E + 1, M):
                sh_block(j, gath_safe[:, (j - 1) * Bs:j * Bs])
        else:
            rank = nc_.sync.partition_id()
            for j in range(E + 1, M):
                rhs2 = work.tile([H, Bs], BF16, tag="rhs2")
                src0 = ((rank + j) % M) * H
                nc_.sync.dma_start(out=rhs2[:],
                                   in_=cc_out[bass.ds(src0, H), :])
                sh_block(j, rhs2[:])
        for bci in range(E + 1, M):
            gsel = (1,) if (HX and bci == E + 1) else (0, 1)
            z_tile(shT_sb[:, bci * H:(bci + 1) * H], 0,
                   d_z[bci * H:(bci + 1) * H, :], bci, groups=gsel,
                   fine=(bci == M - 1))

        if PEER_GATHER:
            # zero the sems so a later execution of this NEFF starts
            # clean; the lsem>=112 drain waits are attached post-schedule
            clr1 = nc_.gpsimd.sem_clear(rsem)
            clr2 = nc_.gpsimd.sem_clear(lsem)

    if PEER_GATHER:
        # cross-core sem waits, invisible to the single-core Tile sim
        rwait_inst.wait_op(rsem, (M - 1) * (16 // M), "sem-ge")
        clr1.wait_op(lsem, (M - 1) * 16, "sem-ge")
        clr2.wait_op(lsem, (M - 1) * 16, "sem-ge")

    nc.compile()
    return nc


_CACHE = {}


def _get_program(bq_val):
    key = round(float(bq_val), 10)
    if key not in _CACHE:
        _CACHE[key] = _build_program(bq_val)
    return _CACHE[key]


def kernel(session_embedding, item_emb, batch, num_graphs,
           W1, b1, W2, b2, q, bq, W3, b3):
    import ml_dtypes
    BF = ml_dtypes.bfloat16

    session = np.ascontiguousarray(np.asarray(session_embedding, np.float32))
    item = np.ascontiguousarray(np.asarray(item_emb, np.float32))
    batch = np.asarray(batch)
    W1 = np.asarray(W1, np.float32)
    b1 = np.asarray(b1, np.float32)
    W2 = np.asarray(W2, np.float32)
    b2 = np.asarray(b2, np.float32)
    q = np.asarray(q, np.float32)
    bq = np.asarray(bq, np.float32)
    W3 = np.asarray(W3, np.float32)
    b3 = np.asarray(b3, np.float32)

    uniform = (session.shape == (N, H) and item.shape == (V, H)
               and batch.shape == (N,)
               and int(num_graphs) == B
               and np.array_equal(batch, np.repeat(np.arange(B), L)))
    if not uniform:
        return _kernel_numpy(session, item, batch, W1, b1, W2, b2,
                             q, bq, W3, b3)

    from concourse.bass_utils import run_bass_kernel_spmd

    nc = _get_program(bq[0])

    # ---- host-side shard prep (index bookkeeping + bf16 casts) ----
    last_idx = np.arange(B) * L + (L - 1)
    v_n = session[last_idx]                       # [B, H]
    vnfT = np.ascontiguousarray(v_n.T.astype(BF))  # [H, B]

    itemT = np.ascontiguousarray(item.T.astype(BF))  # [H, V]
    sessT = session.T.astype(BF)                     # [H, N]

    wp = np.concatenate(
        [W1, W2, W3[:H], W3[H:], q.reshape(H, 1)], axis=1).astype(BF)
    wp = np.ascontiguousarray(wp)
    bp = np.ascontiguousarray(
        np.stack([b1 + b2, b3], axis=1).astype(np.float32))

    def gmap(k, j):
        return (k ^ j) if PEER_GATHER else ((k + j) % M)

    in_maps = []
    for k in range(M):
        nsl = slice(k * Ns, (k + 1) * Ns)
        vnf_rot = np.concatenate(
            [vnfT[:, gmap(k, j) * Bs:(gmap(k, j) + 1) * Bs]
             for j in range(M)], axis=1)
        # extra shard i is the vocab shard whose column-owner skips
        # local row-block i = our rows: v = (k - i) mod M; plus the first
        # half of shard (k-4) whose owner computes only its second half
        icols = [itemT[:, ((k - i) % M) * Vs:(((k - i) % M) + 1) * Vs]
                 for i in range(1 + E)]
        if E == 3:
            v4 = ((k - 4) % M) * Vs
            icols.append(itemT[:, v4:v4 + ZG])
        in_maps.append({
            "xT": np.ascontiguousarray(sessT[:, nsl]),
            "vnf": np.ascontiguousarray(vnf_rot),
            "itemT": np.ascontiguousarray(np.concatenate(icols, axis=1)),
            "wp": wp, "bp": bp,
        })

    res = run_bass_kernel_spmd(nc, in_maps, list(range(M)))

    # un-rotate: core k's local z row-block j holds graphs gmap(k,j)*Bs..
    # blocks j in 1..E come instead from core gmap(k,j)'s d_ze extras.
    z = np.empty((B, V), np.float32)
    half_split = (E == 3)
    for k in range(M):
        zk = res.results[k]["z"]
        for j in [0] + list(range(E + 1, M)):
            gblk = gmap(k, j)
            c0 = ZG if (half_split and j == E + 1) else 0
            z[gblk * Bs:(gblk + 1) * Bs, k * Vs + c0:(k + 1) * Vs] = \
                zk[j * Bs:(j + 1) * Bs, c0:].astype(np.float32)
        if E > 0:
            zek = res.results[k]["ze"]
            for i in range(1, E + 1):
                vblk = (k - i) % M
                z[k * Bs:(k + 1) * Bs, vblk * Vs:(vblk + 1) * Vs] = \
                    zek[:, (i - 1) * Vs:i * Vs].astype(np.float32)
            if half_split:
                v4 = ((k - 4) % M) * Vs
                z[k * Bs:(k + 1) * Bs, v4:v4 + ZG] = \
                    zek[:, E * Vs:E * Vs + ZG].astype(np.float32)
    return z
